# revision 12
# baseline (speedup 1.0000x reference)
"""Trainium2 Bass kernel for a 3-layer PointGNN-style edge-scored message-passing GNN.

Per layer (host-folded weights):
    x' = X@W + b ; B = X@(W Wj) + b Wj ; A = X@(W Wi) + (b Wi + bs1)
    h = relu(A[dst] + B[src]) ; s = sigmoid(h.w2 + b2)
    out[d] = sum_{e: dst=d} s_e * x'[src_e]       (+relu for layers 1,2)

Device (8-core SPMD, dst-partitioned):
  - nodes permuted into nb=240 bins of 128 slots, balanced by in-degree; each
    bin's (edges+self-loops) list padded to uniform g_pad, dst-bin sorted.
  - node phase (own 30 bins only): one matmul chain per bin emits a bf16 row
    table [x' | B | A] (row padded to a 256B-multiple stride); rows staged and
    AllGathered into a full DRAM table [ng, row_w].
  - edge phase (own 30 bins, chunks of <=8 128-edge tiles): two dma_gathers
    per chunk ([x'|B] rows by src, A rows by dst); batched DVE ops (one-hot,
    add, relu*w2, segmented reduce, sigma-scale) compute edge scores; one
    bf16 matmul per 128-edge tile scatter-adds weighted messages per dst bin.
"""

import sys

if "/opt/trn_rl_repo" not in sys.path:
    sys.path.insert(0, "/opt/trn_rl_repo")

import numpy as np
import ml_dtypes

import concourse.bacc as bacc
import concourse.bass as bass  # noqa: F401
import concourse.mybir as mybir
import concourse.tile as tile
from concourse.bass_utils import run_bass_kernel_spmd

F32 = mybir.dt.float32
BF16 = mybir.dt.bfloat16
I16 = mybir.dt.int16
AF = mybir.ActivationFunctionType
ALU = mybir.AluOpType
NPBF = ml_dtypes.bfloat16

P = 128
NCORES = 8
CH = 8          # tiles (128 edges each) per gather chunk / compute group
SIM_MODE = False  # replace collectives with local copies (TimelineSim support)


def _row_w(co):
    """Table row width (elements): [x'|B|A] padded to a 256B-multiple stride."""
    w = 3 * co
    pad = (-w * 2) % 256  # bf16 bytes
    return w + pad // 2


class Cfg:
    def __init__(self, n_real, nbc, dims):
        self.n_real = n_real
        self.nbc = nbc
        self.nb = nbc * NCORES
        self.ng = self.nb * P
        self.dims = dims


CFG = Cfg(30000, 32, [(256, 64), (64, 64), (64, 256)])


# ---------------------------------------------------------------- host prep

def _balance_bins(weight, nb):
    """Assign nodes to nb bins of <=128 slots, balancing sum(weight)."""
    import heapq

    n = weight.shape[0]
    order = np.argsort(-weight, kind="stable")
    bin_of = np.empty(n, np.int32)
    slot_of = np.empty(n, np.int32)
    counts = np.zeros(nb, np.int32)
    heap = [(0, b) for b in range(nb)]
    heapq.heapify(heap)
    for i in order:
        spill = []
        while True:
            load, b = heapq.heappop(heap)
            if counts[b] < P:
                break
            spill.append((load, b))
        for s in spill:
            heapq.heappush(heap, s)
        bin_of[i] = b
        slot_of[i] = counts[b]
        counts[b] += 1
        heapq.heappush(heap, (load + int(weight[i]), b))
    return bin_of, slot_of


def _wrap16(flat_idx):
    n = flat_idx.shape[0]
    a = flat_idx.reshape(n // 16, 16).T.astype(np.int16)
    return np.tile(a, (8, 1))


def _host_prep(x, src, dst, cfg):
    n = cfg.n_real
    loops = np.arange(n, dtype=np.int64)
    src_all = np.concatenate([src, loops])
    dst_all = np.concatenate([dst, loops])

    indeg = np.bincount(dst_all, minlength=n).astype(np.int64)
    bin_of, slot_of = _balance_bins(indeg, cfg.nb)
    g_of = bin_of.astype(np.int64) * P + slot_of

    e_bin = bin_of[dst_all]
    order = np.argsort(e_bin, kind="stable")
    sb = e_bin[order]
    counts = np.bincount(e_bin, minlength=cfg.nb)
    g_pad = int(np.ceil(max(counts.max(), 1) / P) * P)
    starts = np.zeros(cfg.nb, np.int64)
    starts[1:] = np.cumsum(counts)[:-1]
    rank = np.arange(sb.shape[0]) - starts[sb]

    src_g = np.zeros((cfg.nb, g_pad), np.int64)             # pad edges -> row 0
    dst_g = np.zeros((cfg.nb, g_pad), np.int64)             # pad edges -> row 0
    dst_slot = np.full((cfg.nb, g_pad), 255.0, np.float32)  # pad -> no match
    src_g[sb, rank] = g_of[src_all[order]]
    dst_g[sb, rank] = g_of[dst_all[order]]
    dst_slot[sb, rank] = slot_of[dst_all[order]].astype(np.float32)

    nt_e = g_pad // P
    per_core = []
    for c in range(NCORES):
        bins = slice(c * cfg.nbc, (c + 1) * cfg.nbc)
        sg = src_g[bins]
        dg = dst_g[bins]
        srcw = np.concatenate([_wrap16(sg[t]) for t in range(cfg.nbc)], axis=1)
        dstw = np.concatenate([_wrap16(dg[t]) for t in range(cfg.nbc)], axis=1)
        dc = dst_slot[bins]
        dstc = np.concatenate(
            [dc[t].reshape(nt_e, P).T for t in range(cfg.nbc)], axis=1
        ).astype(np.float32)
        per_core.append((srcw, dstw, dstc))

    c_in = cfg.dims[0][0]
    x1t = np.zeros((c_in, cfg.ng), np.float32)
    x1t[:, g_of] = x.T
    return g_of, g_pad, per_core, x1t


def _fuse_weights(ws, cfg):
    out = []
    for li, (ci, co) in enumerate(cfg.dims, start=1):
        wl = ws[f"w_lin{li}"].astype(np.float64)
        bl = ws[f"b_lin{li}"].astype(np.float64)
        ws1 = ws[f"w_s1_{li}"].astype(np.float64)
        bs1 = ws[f"b_s1_{li}"].astype(np.float64)
        ws2 = ws[f"w_s2_{li}"].astype(np.float64)
        bs2 = ws[f"b_s2_{li}"].astype(np.float64)
        wi, wj = ws1[:co], ws1[co:]
        wmat = np.zeros((ci + 1, 3 * co), np.float32)
        wmat[:ci, :co] = wl
        wmat[ci, :co] = bl
        wmat[:ci, co : 2 * co] = wl @ wj
        wmat[ci, co : 2 * co] = bl @ wj
        wmat[:ci, 2 * co :] = wl @ wi
        wmat[ci, 2 * co :] = bl @ wi + bs1
        out.append(dict(wmat=wmat, w2=ws2[:, 0].astype(np.float32), b2=np.float32(bs2[0])))
    return out


# ---------------------------------------------------------------- program

def _build_program(cfg, g_pad):
    nbc, ng = cfg.nbc, cfg.ng
    nt_e = g_pad // P
    dims = cfg.dims
    nl = len(dims)
    c_in1 = dims[0][0]
    co_last = dims[-1][1]
    n_loc = nbc * P
    k1 = c_in1 // P
    chunks = [(j0, min(CH, nt_e - j0)) for j0 in range(0, nt_e, CH)]
    co_max = max(d[1] for d in dims)
    rw_max = max(_row_w(d[1]) for d in dims)

    # f32 const blob columns: b2 per layer + ones_row (row 0)
    c_b2 = [0, 1, 2]
    c_ones = 3
    cf_cols = 3 + P
    # bf16 const blob columns: iota row + w2rep per layer
    c_iota = 0
    cb_off = []
    off = P
    for l in range(nl):
        cb_off.append(off)
        off += dims[l][1]
    cb_cols = off

    nc = bacc.Bacc(
        "TRN2",
        target_bir_lowering=False,
        debug=False,
        num_devices=NCORES,
        num_swdge_queues=2,
    )

    xt_d = nc.dram_tensor("xt", [c_in1, n_loc], F32, kind="ExternalInput")
    cstf_d = nc.dram_tensor("cstf", [P, cf_cols], F32, kind="ExternalInput")
    cstb_d = nc.dram_tensor("cstb", [P, cb_cols], BF16, kind="ExternalInput")
    srcw_d = nc.dram_tensor("srcw", [P, nbc * g_pad // 16], I16, kind="ExternalInput")
    dstw_d = nc.dram_tensor("dstw", [P, nbc * g_pad // 16], I16, kind="ExternalInput")
    dstc_d = nc.dram_tensor("dstc", [P, nbc * nt_e], BF16, kind="ExternalInput")
    w_d = [
        nc.dram_tensor(f"w{l + 1}", [dims[l][0] + 1, 3 * dims[l][1]], F32, kind="ExternalInput")
        for l in range(nl)
    ]
    out_d = nc.dram_tensor("out", [n_loc, co_last], F32, kind="ExternalOutput")

    with tile.TileContext(nc) as tc:
        with (
            tc.tile_pool(name="cst", bufs=1) as cpool,
            tc.tile_pool(name="persist", bufs=1) as ppool,
            tc.tile_pool(name="stage", bufs=1) as spool,
            tc.tile_pool(name="work", bufs=3) as wpool,
            tc.tile_pool(name="gath", bufs=3) as gpool,
            tc.tile_pool(name="nps", bufs=2, space="PSUM") as npspool,
            tc.tile_pool(name="acc", bufs=2, space="PSUM") as accpool,
            tc.tile_pool(name="dram", bufs=1, space="DRAM") as dpool,
        ):
            # ---------------- constants
            cstf = cpool.tile([P, cf_cols], F32)
            nc.sync.dma_start(cstf[:], cstf_d[:])
            cstb = cpool.tile([P, cb_cols], BF16)
            nc.sync.dma_start(cstb[:], cstb_d[:])
            srcw = cpool.tile([P, nbc * g_pad // 16], I16)
            nc.sync.dma_start(srcw[:], srcw_d[:])
            dstw = cpool.tile([P, nbc * g_pad // 16], I16)
            nc.sync.dma_start(dstw[:], dstw_d[:])
            dstc = cpool.tile([P, nbc * nt_e], BF16)
            nc.sync.dma_start(dstc[:], dstc_d[:])
            wfeat = []
            wbias = []
            for l in range(nl):
                ci_l, co_l = dims[l]
                nk = (ci_l + P - 1) // P
                chs = []
                for k in range(nk):
                    r0, r1 = k * P, min((k + 1) * P, ci_l)
                    w_t = cpool.tile([r1 - r0, 3 * co_l], F32, tag=f"w{l}_{k}")
                    nc.sync.dma_start(w_t[:], w_d[l][:][r0:r1, :])
                    chs.append(w_t)
                wb = cpool.tile([1, 3 * co_l], F32, tag=f"w{l}_b")
                nc.sync.dma_start(wb[:], w_d[l][:][ci_l : ci_l + 1, :])
                wfeat.append(chs)
                wbias.append(wb)

            iota_free = cstb[:, c_iota : c_iota + P]
            ones_row = cstf[0:1, c_ones : c_ones + P]

            # ---------------- persistent SBUF
            xloc = ppool.tile([64, n_loc], F32, tag="xloc")
            tstage = spool.tile([P, nbc * rw_max], BF16, tag="tstage")

            # ---------------- DRAM internals
            ag_in = [
                dpool.tile(
                    [n_loc, _row_w(dims[l][1])], BF16, tag=f"agin{l}", name=f"agin{l}"
                )
                for l in range(nl)
            ]
            tables = [
                dpool.tile(
                    [ng, _row_w(dims[l][1])],
                    BF16,
                    tag=f"table{l}",
                    name=f"table{l}",
                    addr_space="Local" if SIM_MODE else "Shared",
                )
                for l in range(nl)
            ]

            for l in range(nl):
                ci, co = dims[l]
                rw = _row_w(co)

                # ======== node phase: own bins -> [x'|B|A] rows
                for t in range(nbc):
                    cols = slice(t * P, (t + 1) * P)
                    if l == 0:
                        xa = wpool.tile([P, k1 * P], F32, tag="xa")
                        xa3 = xa[:].rearrange("p (c n) -> p c n", c=k1)
                        nc.sync.dma_start(
                            xa3,
                            xt_d[:, cols].rearrange("(c p) n -> p c n", p=P),
                        )
                        kch = [(xa3[:, k, :], wfeat[l][k]) for k in range(k1)]
                    else:
                        kch = [(xloc[:, cols], wfeat[l][0])]
                    if 3 * co <= 512:
                        ps = npspool.tile([P, 512], F32, space="PSUM", tag="nps_a")
                        parts = [(ps, 0, 3 * co)]
                    else:
                        ps1 = npspool.tile([P, 512], F32, space="PSUM", tag="nps_a")
                        ps2 = npspool.tile([P, 256], F32, space="PSUM", tag="nps_b")
                        parts = [(ps1, 0, 2 * co), (ps2, 2 * co, 3 * co)]
                    for pst, cc0, cc1 in parts:
                        for k, (kc, wt) in enumerate(kch):
                            nc.tensor.matmul(
                                out=pst[:, 0 : cc1 - cc0],
                                lhsT=kc,
                                rhs=wt[:, cc0:cc1],
                                start=(k == 0),
                                stop=False,
                            )
                        nc.tensor.matmul(
                            out=pst[:, 0 : cc1 - cc0],
                            lhsT=ones_row,
                            rhs=wbias[l][0:1, cc0:cc1],
                            start=False,
                            stop=True,
                        )
                    ts0 = t * rw
                    if len(parts) == 1:
                        ps = parts[0][0]
                        nc.scalar.activation(
                            out=tstage[:, ts0 : ts0 + 2 * co],
                            in_=ps[:, 0 : 2 * co],
                            func=AF.Copy,
                        )
                        nc.scalar.activation(
                            out=tstage[:, ts0 + 2 * co : ts0 + 3 * co],
                            in_=ps[:, 2 * co : 3 * co],
                            func=AF.Copy,
                        )
                        if rw > 3 * co:
                            nc.scalar.activation(
                                out=tstage[:, ts0 + 3 * co : ts0 + rw],
                                in_=ps[:, 2 * co : 2 * co + (rw - 3 * co)],
                                func=AF.Copy,
                            )
                    else:
                        nc.scalar.activation(
                            out=tstage[:, ts0 : ts0 + 2 * co],
                            in_=parts[0][0][:, 0 : 2 * co],
                            func=AF.Copy,
                        )
                        nc.scalar.activation(
                            out=tstage[:, ts0 + 2 * co : ts0 + 3 * co],
                            in_=parts[1][0][:, 0:co],
                            func=AF.Copy,
                        )

                # stage -> DRAM rows, then AllGather into the full table
                nc.sync.dma_start(
                    ag_in[l][:].rearrange("(t p) c -> p t c", p=P),
                    tstage[:, 0 : nbc * rw].rearrange("p (t c) -> p t c", c=rw),
                )
                if SIM_MODE:
                    for r in range(NCORES):
                        nc.sync.dma_start(
                            tables[l][:][r * n_loc : (r + 1) * n_loc, :], ag_in[l][:]
                        )
                else:
                    nc.gpsimd.collective_compute(
                        "AllGather",
                        ALU.bypass,
                        replica_groups=[list(range(NCORES))],
                        ins=[ag_in[l].opt()],
                        outs=[tables[l].opt()],
                    )

                # ======== edge phase: own bins
                w2rep = cstb[:, cb_off[l] : cb_off[l] + co]
                b2col = cstf[:, c_b2[l] : c_b2[l] + 1]
                for t in range(nbc):
                    o_full = accpool.tile([P, 256], F32, space="PSUM", tag="o_ps")
                    o_ps = o_full[0:64, 0:P] if l < nl - 1 else o_full[:, 0:co]
                    first_mm = True
                    for ci_ch, (j0, hn) in enumerate(chunks):
                        gt0 = t * nt_e + j0
                        gbuf = gpool.tile([P, CH * 2 * co], BF16, tag="gbuf")
                        g3 = gbuf[:, 0 : hn * 2 * co].rearrange(
                            "p (j d) -> p j d", d=2 * co
                        )
                        nc.gpsimd.dma_gather(
                            out_ap=g3,
                            in_ap=tables[l][:][:, 0 : 2 * co],
                            idxs_ap=srcw[:, gt0 * 8 : (gt0 + hn) * 8],
                            num_idxs=hn * P,
                            num_idxs_reg=hn * P,
                            elem_size=2 * co,
                            elem_step=rw,
                            queue_num=ci_ch % 2,
                        )
                        a_el = max(co, P)  # elem must be a 256B multiple (bf16)
                        abuf = gpool.tile([P, CH * a_el], BF16, tag="abuf")
                        ag3 = abuf[:, 0 : hn * a_el].rearrange(
                            "p (j d) -> p j d", d=a_el
                        )
                        nc.gpsimd.dma_gather(
                            out_ap=ag3,
                            in_ap=tables[l][:][:, 2 * co : 2 * co + a_el],
                            idxs_ap=dstw[:, gt0 * 8 : (gt0 + hn) * 8],
                            num_idxs=hn * P,
                            num_idxs_reg=hn * P,
                            elem_size=a_el,
                            elem_step=rw,
                            queue_num=(ci_ch + 1) % 2,
                        )
                        a3 = ag3[:, :, 0:co]
                        # one-hot [e, slot] per tile for the scatter
                        oh = wpool.tile([P, CH * P], BF16, tag="oh")
                        oh3 = oh[:, 0 : hn * P].rearrange("p (j s) -> p j s", s=P)
                        nc.vector.tensor_tensor(
                            out=oh3,
                            in0=dstc[:, t * nt_e + j0 : t * nt_e + j0 + hn]
                            .rearrange("p (j o) -> p j o", o=1)
                            .to_broadcast([P, hn, P]),
                            in1=iota_free.rearrange("p (o s) -> p o s", o=1)
                            .to_broadcast([P, hn, P]),
                            op=ALU.is_equal,
                        )
                        # pre-activation, relu * w2, segmented row-sum
                        tmp = wpool.tile([P, CH * co_max], BF16, tag="tmp")
                        t3 = tmp[:, 0 : hn * co].rearrange("p (j d) -> p j d", d=co)
                        nc.vector.tensor_tensor(
                            out=t3,
                            in0=a3,
                            in1=g3[:, :, co : 2 * co],
                            op=ALU.add,
                        )
                        tmp2 = wpool.tile([P, CH * co_max], BF16, tag="tmp2")
                        nc.vector.scalar_tensor_tensor(
                            out=tmp2[:, 0 : hn * co].rearrange("p (j d) -> p j d", d=co),
                            in0=t3,
                            scalar=0.0,
                            in1=w2rep.rearrange("p (o d) -> p o d", o=1)
                            .to_broadcast([P, hn, co]),
                            op0=ALU.max,
                            op1=ALU.mult,
                        )
                        spre = wpool.tile([P, CH], F32, tag="spre")
                        nc.vector.tensor_reduce(
                            out=spre[:, 0:hn],
                            in_=tmp2[:, 0 : hn * co].rearrange("p (j d) -> p j d", d=co),
                            axis=mybir.AxisListType.X,
                            op=ALU.add,
                        )
                        ssig = wpool.tile([P, CH], BF16, tag="ssig")
                        nc.scalar.activation(
                            out=ssig[:, 0:hn],
                            in_=spre[:, 0:hn],
                            func=AF.Sigmoid,
                            bias=b2col,
                        )
                        xs = wpool.tile([P, CH * co_max], BF16, tag="xs")
                        nc.vector.tensor_tensor(
                            out=xs[:, 0 : hn * co].rearrange("p (j d) -> p j d", d=co),
                            in0=g3[:, :, 0:co],
                            in1=ssig[:, 0:hn]
                            .rearrange("p (j o) -> p j o", o=1)
                            .to_broadcast([P, hn, co]),
                            op=ALU.mult,
                        )
                        last_ch = ci_ch == len(chunks) - 1
                        for u in range(hn):
                            if l < nl - 1:
                                nc.tensor.matmul(
                                    out=o_ps,
                                    lhsT=xs[:, u * co : (u + 1) * co],
                                    rhs=oh3[:, u, :],
                                    start=first_mm,
                                    stop=last_ch and (u == hn - 1),
                                )
                            else:
                                nc.tensor.matmul(
                                    out=o_ps,
                                    lhsT=oh3[:, u, :],
                                    rhs=xs[:, u * co : (u + 1) * co],
                                    start=first_mm,
                                    stop=last_ch and (u == hn - 1),
                                )
                            first_mm = False
                    if l < nl - 1:
                        nc.scalar.activation(
                            out=xloc[:, t * P : (t + 1) * P], in_=o_ps, func=AF.Relu
                        )
                    else:
                        ostg = wpool.tile([P, co], F32, tag="ostg")
                        nc.scalar.activation(out=ostg[:], in_=o_ps, func=AF.Copy)
                        nc.sync.dma_start(out_d[t * P : (t + 1) * P, :], ostg[:])

    nc.compile()
    return nc


# ---------------------------------------------------------------- driver

_PROG_CACHE = {}


def _make_in_maps(inputs, cfg, g_pad, per_core, x1t, fw):
    nbc, nl = cfg.nbc, len(cfg.dims)
    n_loc = nbc * P
    cf_cols = 3 + P
    cstf = np.zeros((P, cf_cols), np.float32)
    for l in range(nl):
        cstf[:, l] = fw[l]["b2"]
    cstf[0, 3 : 3 + P] = 1.0
    cb_cols = P + sum(d[1] for d in cfg.dims)
    cstb = np.zeros((P, cb_cols), NPBF)
    cstb[:, 0:P] = np.arange(P, dtype=np.float32)[None, :].astype(NPBF)
    off = P
    for l in range(nl):
        cstb[:, off : off + cfg.dims[l][1]] = fw[l]["w2"][None, :].astype(NPBF)
        off += cfg.dims[l][1]

    in_maps = []
    for c in range(NCORES):
        srcw, dstw, dstc_a = per_core[c]
        in_maps.append(
            {
                "xt": np.ascontiguousarray(x1t[:, c * n_loc : (c + 1) * n_loc]),
                "cstf": cstf,
                "cstb": cstb,
                "srcw": srcw,
                "dstw": dstw,
                "dstc": dstc_a.astype(NPBF),
                **{f"w{l + 1}": fw[l]["wmat"] for l in range(nl)},
            }
        )
    return in_maps


def _run(inputs, cfg, trace=False):
    x = np.ascontiguousarray(np.asarray(inputs["x"], dtype=np.float32))
    ei = np.asarray(inputs["edge_index"]).astype(np.int64)
    src, dst = ei[0], ei[1]

    g_of, g_pad, per_core, x1t = _host_prep(x, src, dst, cfg)
    fw = _fuse_weights(inputs, cfg)

    key = (cfg.n_real, cfg.nbc, g_pad)
    if key not in _PROG_CACHE:
        _PROG_CACHE[key] = _build_program(cfg, g_pad)
    nc = _PROG_CACHE[key]

    in_maps = _make_in_maps(inputs, cfg, g_pad, per_core, x1t, fw)
    res = run_bass_kernel_spmd(nc, in_maps, core_ids=list(range(NCORES)), trace=trace)

    n_loc = cfg.nbc * P
    full = np.empty((cfg.ng, cfg.dims[-1][1]), np.float32)
    for c in range(NCORES):
        full[c * n_loc : (c + 1) * n_loc] = res.results[c]["out"]
    out = full[g_of]
    return out, res


def kernel(**inputs) -> np.ndarray:
    out, _ = _run(inputs, CFG, trace=False)
    return out


# revision 16
# speedup vs baseline: 1.8354x; 1.8354x over previous
"""Trainium2 Bass kernel for a 3-layer PointGNN-style edge-scored message-passing GNN.

Per layer (host-folded weights):
    x' = X@W + b ; B = X@(W Wj) + b Wj ; A = X@(W Wi) + (b Wi + bs1)
    h = relu(A[dst] + B[src]) ; s = sigmoid(h.w2 + b2)
    out[d] = sum_{e: dst=d} s_e * x'[src_e]       (+relu for layers 1,2)

Device (8-core SPMD, dst-partitioned):
  - nodes permuted into nb=256 bins of 128 slots, balanced by in-degree; each
    bin's (edges+self-loops) list padded to uniform g_pad (16 tiles),
    dst-bin sorted. ng=32768 so node ids fit int16 gather indices.
  - node phase (own 32 bins, fp32 matmuls for precision): one chain per bin
    emits [x' | B | A]; [x'|B] rows (bf16) staged and AllGathered into a full
    DRAM table [ng, 2co]; A rows kept in a core-local DRAM table (dst is
    always core-local, so A never rides the collective).
  - edge phase (own 32 bins, chunks of 8 128-edge tiles): two dma_gathers per
    chunk ([x'|B] by src from the shared table, A by local dst); batched bf16
    DVE ops (one-hot, add, relu*w2, segmented reduce, sigma-scale) compute
    edge scores; one bf16 matmul per 128-edge tile scatter-adds weighted
    messages into the dst bin's PSUM accumulator (sigma folded into the
    messages for co=64 layers, into the one-hot for the wide co=256 layer).
"""

import sys

if "/opt/trn_rl_repo" not in sys.path:
    sys.path.insert(0, "/opt/trn_rl_repo")

import numpy as np
import ml_dtypes

import concourse.bacc as bacc
import concourse.bass as bass  # noqa: F401
import concourse.mybir as mybir
import concourse.tile as tile
from concourse.bass_utils import run_bass_kernel_spmd

F32 = mybir.dt.float32
BF16 = mybir.dt.bfloat16
I16 = mybir.dt.int16
AF = mybir.ActivationFunctionType
ALU = mybir.AluOpType
NPBF = ml_dtypes.bfloat16

P = 128
NCORES = 8
CH = 8          # tiles (128 edges each) per gather chunk / compute group
SIM_MODE = False  # replace collectives with local copies (TimelineSim support)


def _a_w(co):
    """Local A-table row width (elements): A padded to a 256B-multiple stride."""
    w = co
    pad = (-w * 2) % 256  # bf16 bytes
    return w + pad // 2


class Cfg:
    def __init__(self, n_real, nbc, dims):
        self.n_real = n_real
        self.nbc = nbc
        self.nb = nbc * NCORES
        self.ng = self.nb * P
        self.dims = dims


CFG = Cfg(30000, 32, [(256, 64), (64, 64), (64, 256)])


# ---------------------------------------------------------------- host prep

def _balance_bins(weight, nb):
    """Assign nodes to nb bins of <=128 slots, balancing sum(weight)."""
    import heapq

    n = weight.shape[0]
    order = np.argsort(-weight, kind="stable")
    bin_of = np.empty(n, np.int32)
    slot_of = np.empty(n, np.int32)
    counts = np.zeros(nb, np.int32)
    heap = [(0, b) for b in range(nb)]
    heapq.heapify(heap)
    for i in order:
        spill = []
        while True:
            load, b = heapq.heappop(heap)
            if counts[b] < P:
                break
            spill.append((load, b))
        for s in spill:
            heapq.heappush(heap, s)
        bin_of[i] = b
        slot_of[i] = counts[b]
        counts[b] += 1
        heapq.heappush(heap, (load + int(weight[i]), b))
    return bin_of, slot_of


def _wrap16(flat_idx):
    n = flat_idx.shape[0]
    a = flat_idx.reshape(n // 16, 16).T.astype(np.int16)
    return np.tile(a, (8, 1))


def _host_prep(x, src, dst, cfg):
    n = cfg.n_real
    loops = np.arange(n, dtype=np.int64)
    src_all = np.concatenate([src, loops])
    dst_all = np.concatenate([dst, loops])

    indeg = np.bincount(dst_all, minlength=n).astype(np.int64)
    bin_of, slot_of = _balance_bins(indeg, cfg.nb)
    g_of = bin_of.astype(np.int64) * P + slot_of

    e_bin = bin_of[dst_all]
    order = np.argsort(e_bin, kind="stable")
    sb = e_bin[order]
    counts = np.bincount(e_bin, minlength=cfg.nb)
    g_pad = int(np.ceil(max(counts.max(), 1) / P) * P)
    starts = np.zeros(cfg.nb, np.int64)
    starts[1:] = np.cumsum(counts)[:-1]
    rank = np.arange(sb.shape[0]) - starts[sb]

    src_g = np.zeros((cfg.nb, g_pad), np.int64)             # pad edges -> row 0
    dst_g = np.zeros((cfg.nb, g_pad), np.int64)             # pad edges -> row 0
    dst_slot = np.full((cfg.nb, g_pad), 255.0, np.float32)  # pad -> no match
    src_g[sb, rank] = g_of[src_all[order]]
    dst_g[sb, rank] = g_of[dst_all[order]]
    dst_slot[sb, rank] = slot_of[dst_all[order]].astype(np.float32)

    nt_e = g_pad // P
    per_core = []
    for c in range(NCORES):
        bins = slice(c * cfg.nbc, (c + 1) * cfg.nbc)
        sg = src_g[bins]
        dg = dst_g[bins] - c * cfg.nbc * P
        dg[dst_slot[bins] == 255.0] = 0
        srcw = np.concatenate([_wrap16(sg[t]) for t in range(cfg.nbc)], axis=1)
        dstw = np.concatenate([_wrap16(dg[t]) for t in range(cfg.nbc)], axis=1)
        dc = dst_slot[bins]
        dstc = np.concatenate(
            [dc[t].reshape(nt_e, P).T for t in range(cfg.nbc)], axis=1
        ).astype(np.float32)
        per_core.append((srcw, dstw, dstc))

    c_in = cfg.dims[0][0]
    x1t = np.zeros((c_in, cfg.ng), np.float32)
    x1t[:, g_of] = x.T
    return g_of, g_pad, per_core, x1t


def _fuse_weights(ws, cfg):
    out = []
    for li, (ci, co) in enumerate(cfg.dims, start=1):
        wl = ws[f"w_lin{li}"].astype(np.float64)
        bl = ws[f"b_lin{li}"].astype(np.float64)
        ws1 = ws[f"w_s1_{li}"].astype(np.float64)
        bs1 = ws[f"b_s1_{li}"].astype(np.float64)
        ws2 = ws[f"w_s2_{li}"].astype(np.float64)
        bs2 = ws[f"b_s2_{li}"].astype(np.float64)
        wi, wj = ws1[:co], ws1[co:]
        wmat = np.zeros((ci + 1, 3 * co), np.float32)
        wmat[:ci, :co] = wl
        wmat[ci, :co] = bl
        wmat[:ci, co : 2 * co] = wl @ wj
        wmat[ci, co : 2 * co] = bl @ wj
        wmat[:ci, 2 * co :] = wl @ wi
        wmat[ci, 2 * co :] = bl @ wi + bs1
        out.append(dict(wmat=wmat, w2=ws2[:, 0].astype(np.float32), b2=np.float32(bs2[0])))
    return out


# ---------------------------------------------------------------- program

def _build_program(cfg, g_pad):
    nbc, ng = cfg.nbc, cfg.ng
    nt_e = g_pad // P
    dims = cfg.dims
    nl = len(dims)
    c_in1 = dims[0][0]
    co_last = dims[-1][1]
    n_loc = nbc * P
    k1 = c_in1 // P
    chunks = [(j0, min(CH, nt_e - j0)) for j0 in range(0, nt_e, CH)]
    co_max = max(d[1] for d in dims)
    aw_max = max(_a_w(d[1]) for d in dims)

    # f32 const blob columns: b2 per layer + ones_row (row 0)
    c_b2 = [0, 1, 2]
    c_ones = 3
    cf_cols = 3 + P
    # bf16 const blob columns: iota row + w2rep per layer
    c_iota = 0
    cb_off = []
    off = P
    for l in range(nl):
        cb_off.append(off)
        off += dims[l][1]
    cb_cols = off

    nc = bacc.Bacc(
        "TRN2",
        target_bir_lowering=False,
        debug=False,
        num_devices=NCORES,
        num_swdge_queues=2,
    )

    xt_d = nc.dram_tensor("xt", [c_in1, n_loc], F32, kind="ExternalInput")
    cstf_d = nc.dram_tensor("cstf", [P, cf_cols], F32, kind="ExternalInput")
    cstb_d = nc.dram_tensor("cstb", [P, cb_cols], BF16, kind="ExternalInput")
    srcw_d = nc.dram_tensor("srcw", [P, nbc * g_pad // 16], I16, kind="ExternalInput")
    dstw_d = nc.dram_tensor("dstw", [P, nbc * g_pad // 16], I16, kind="ExternalInput")
    dstc_d = nc.dram_tensor("dstc", [P, nbc * nt_e], BF16, kind="ExternalInput")
    w_d = [
        nc.dram_tensor(f"w{l + 1}", [dims[l][0] + 1, 3 * dims[l][1]], F32, kind="ExternalInput")
        for l in range(nl)
    ]
    out_d = nc.dram_tensor("out", [n_loc, co_last], F32, kind="ExternalOutput")

    with tile.TileContext(nc) as tc:
        with (
            tc.tile_pool(name="cst", bufs=1) as cpool,
            tc.tile_pool(name="persist", bufs=1) as ppool,
            tc.tile_pool(name="stage", bufs=1) as spool,
            tc.tile_pool(name="work", bufs=3) as wpool,
            tc.tile_pool(name="gath", bufs=3) as gpool,
            tc.tile_pool(name="nps", bufs=2, space="PSUM") as npspool,
            tc.tile_pool(name="acc", bufs=2, space="PSUM") as accpool,
            tc.tile_pool(name="dram", bufs=1, space="DRAM") as dpool,
        ):
            # ---------------- constants
            cstf = cpool.tile([P, cf_cols], F32)
            nc.sync.dma_start(cstf[:], cstf_d[:])
            cstb = cpool.tile([P, cb_cols], BF16)
            nc.sync.dma_start(cstb[:], cstb_d[:])
            srcw = cpool.tile([P, nbc * g_pad // 16], I16)
            nc.sync.dma_start(srcw[:], srcw_d[:])
            dstw = cpool.tile([P, nbc * g_pad // 16], I16)
            nc.sync.dma_start(dstw[:], dstw_d[:])
            dstc = cpool.tile([P, nbc * nt_e], BF16)
            nc.sync.dma_start(dstc[:], dstc_d[:])
            wfeat = []
            wbias = []
            for l in range(nl):
                ci_l, co_l = dims[l]
                nk = (ci_l + P - 1) // P
                chs = []
                for k in range(nk):
                    r0, r1 = k * P, min((k + 1) * P, ci_l)
                    w_t = cpool.tile([r1 - r0, 3 * co_l], F32, tag=f"w{l}_{k}")
                    nc.sync.dma_start(w_t[:], w_d[l][:][r0:r1, :])
                    chs.append(w_t)
                wb = cpool.tile([1, 3 * co_l], F32, tag=f"w{l}_b")
                nc.sync.dma_start(wb[:], w_d[l][:][ci_l : ci_l + 1, :])
                wfeat.append(chs)
                wbias.append(wb)

            iota_free = cstb[:, c_iota : c_iota + P]
            ones_row = cstf[0:1, c_ones : c_ones + P]

            # ---------------- persistent SBUF
            xloc = ppool.tile([64, n_loc], F32, tag="xloc")
            tstage = spool.tile([P, nbc * 2 * co_max], BF16, tag="tstage")
            astage = spool.tile([P, nbc * aw_max], BF16, tag="astage")

            # ---------------- DRAM internals
            ag_in = [
                dpool.tile(
                    [n_loc, 2 * dims[l][1]], BF16, tag=f"agin{l}", name=f"agin{l}"
                )
                for l in range(nl)
            ]
            tables = [
                dpool.tile(
                    [ng, 2 * dims[l][1]],
                    BF16,
                    tag=f"table{l}",
                    name=f"table{l}",
                    addr_space="Local" if SIM_MODE else "Shared",
                )
                for l in range(nl)
            ]
            atabs = [
                dpool.tile(
                    [n_loc, _a_w(dims[l][1])], BF16, tag=f"atab{l}", name=f"atab{l}"
                )
                for l in range(nl)
            ]

            for l in range(nl):
                ci, co = dims[l]
                aw = _a_w(co)

                # ======== node phase: own bins -> [x'|B|A] rows
                for t in range(nbc):
                    cols = slice(t * P, (t + 1) * P)
                    if l == 0:
                        xa = wpool.tile([P, k1 * P], F32, tag="xa")
                        xa3 = xa[:].rearrange("p (c n) -> p c n", c=k1)
                        nc.sync.dma_start(
                            xa3,
                            xt_d[:, cols].rearrange("(c p) n -> p c n", p=P),
                        )
                        kch = [(xa3[:, k, :], wfeat[l][k]) for k in range(k1)]
                    else:
                        kch = [(xloc[:, cols], wfeat[l][0])]
                    if 3 * co <= 512:
                        ps = npspool.tile([P, 512], F32, space="PSUM", tag="nps_a")
                        parts = [(ps, 0, 3 * co)]
                    else:
                        ps1 = npspool.tile([P, 512], F32, space="PSUM", tag="nps_a")
                        ps2 = npspool.tile([P, 256], F32, space="PSUM", tag="nps_b")
                        parts = [(ps1, 0, 2 * co), (ps2, 2 * co, 3 * co)]
                    for pst, cc0, cc1 in parts:
                        for k, (kc, wt) in enumerate(kch):
                            nc.tensor.matmul(
                                out=pst[:, 0 : cc1 - cc0],
                                lhsT=kc,
                                rhs=wt[:, cc0:cc1],
                                start=(k == 0),
                                stop=False,
                            )
                        nc.tensor.matmul(
                            out=pst[:, 0 : cc1 - cc0],
                            lhsT=ones_row,
                            rhs=wbias[l][0:1, cc0:cc1],
                            start=False,
                            stop=True,
                        )
                    ps_a, a_lo = (
                        (parts[0][0], 2 * co) if len(parts) == 1 else (parts[1][0], 0)
                    )
                    nc.scalar.activation(
                        out=tstage[:, t * 2 * co : (t + 1) * 2 * co],
                        in_=parts[0][0][:, 0 : 2 * co],
                        func=AF.Copy,
                    )
                    nc.scalar.activation(
                        out=astage[:, t * aw : t * aw + co],
                        in_=ps_a[:, a_lo : a_lo + co],
                        func=AF.Copy,
                    )
                    if aw > co:
                        nc.scalar.activation(
                            out=astage[:, t * aw + co : (t + 1) * aw],
                            in_=ps_a[:, a_lo : a_lo + co],
                            func=AF.Copy,
                        )

                # stage -> DRAM rows, then AllGather into the full table
                nc.sync.dma_start(
                    ag_in[l][:].rearrange("(t p) c -> p t c", p=P),
                    tstage[:, 0 : nbc * 2 * co].rearrange("p (t c) -> p t c", c=2 * co),
                )
                nc.sync.dma_start(
                    atabs[l][:].rearrange("(t p) c -> p t c", p=P),
                    astage[:, 0 : nbc * aw].rearrange("p (t c) -> p t c", c=aw),
                )
                if SIM_MODE:
                    for r in range(NCORES):
                        nc.sync.dma_start(
                            tables[l][:][r * n_loc : (r + 1) * n_loc, :], ag_in[l][:]
                        )
                else:
                    nc.gpsimd.collective_compute(
                        "AllGather",
                        ALU.bypass,
                        replica_groups=[list(range(NCORES))],
                        ins=[ag_in[l].opt()],
                        outs=[tables[l].opt()],
                    )

                # ======== edge phase: own bins
                w2rep = cstb[:, cb_off[l] : cb_off[l] + co]
                b2col = cstf[:, c_b2[l] : c_b2[l] + 1]
                for t in range(nbc):
                    o_full = accpool.tile([P, 256], F32, space="PSUM", tag="o_ps")
                    o_ps = o_full[0:64, 0:P] if l < nl - 1 else o_full[:, 0:co]
                    first_mm = True
                    for ci_ch, (j0, hn) in enumerate(chunks):
                        gt0 = t * nt_e + j0
                        gbuf = gpool.tile([P, CH * 2 * co], BF16, tag="gbuf")
                        g3 = gbuf[:, 0 : hn * 2 * co].rearrange(
                            "p (j d) -> p j d", d=2 * co
                        )
                        nc.gpsimd.dma_gather(
                            out_ap=g3,
                            in_ap=tables[l][:],
                            idxs_ap=srcw[:, gt0 * 8 : (gt0 + hn) * 8],
                            num_idxs=hn * P,
                            num_idxs_reg=hn * P,
                            elem_size=2 * co,
                            queue_num=ci_ch % 2,
                        )
                        abuf = gpool.tile([P, CH * aw], BF16, tag="abuf")
                        ag3 = abuf[:, 0 : hn * aw].rearrange(
                            "p (j d) -> p j d", d=aw
                        )
                        nc.gpsimd.dma_gather(
                            out_ap=ag3,
                            in_ap=atabs[l][:],
                            idxs_ap=dstw[:, gt0 * 8 : (gt0 + hn) * 8],
                            num_idxs=hn * P,
                            num_idxs_reg=hn * P,
                            elem_size=aw,
                            queue_num=(ci_ch + 1) % 2,
                        )
                        a3 = ag3[:, :, 0:co]
                        # one-hot [e, slot] per tile for the scatter
                        oh = wpool.tile([P, CH * P], BF16, tag="oh")
                        oh3 = oh[:, 0 : hn * P].rearrange("p (j s) -> p j s", s=P)
                        nc.vector.tensor_tensor(
                            out=oh3,
                            in0=dstc[:, t * nt_e + j0 : t * nt_e + j0 + hn]
                            .rearrange("p (j o) -> p j o", o=1)
                            .to_broadcast([P, hn, P]),
                            in1=iota_free.rearrange("p (o s) -> p o s", o=1)
                            .to_broadcast([P, hn, P]),
                            op=ALU.is_equal,
                        )
                        # pre-activation, relu * w2, segmented row-sum
                        tmp = wpool.tile([P, CH * co_max], BF16, tag="tmp")
                        t3 = tmp[:, 0 : hn * co].rearrange("p (j d) -> p j d", d=co)
                        nc.vector.tensor_tensor(
                            out=t3,
                            in0=a3,
                            in1=g3[:, :, co : 2 * co],
                            op=ALU.add,
                        )
                        tmp2 = wpool.tile([P, CH * co_max], BF16, tag="tmp2")
                        nc.vector.scalar_tensor_tensor(
                            out=tmp2[:, 0 : hn * co].rearrange("p (j d) -> p j d", d=co),
                            in0=t3,
                            scalar=0.0,
                            in1=w2rep.rearrange("p (o d) -> p o d", o=1)
                            .to_broadcast([P, hn, co]),
                            op0=ALU.max,
                            op1=ALU.mult,
                        )
                        spre = wpool.tile([P, CH], F32, tag="spre")
                        nc.vector.tensor_reduce(
                            out=spre[:, 0:hn],
                            in_=tmp2[:, 0 : hn * co].rearrange("p (j d) -> p j d", d=co),
                            axis=mybir.AxisListType.X,
                            op=ALU.add,
                        )
                        ssig = wpool.tile([P, CH], BF16, tag="ssig")
                        nc.scalar.activation(
                            out=ssig[:, 0:hn],
                            in_=spre[:, 0:hn],
                            func=AF.Sigmoid,
                            bias=b2col,
                        )
                        last_ch = ci_ch == len(chunks) - 1
                        if l < nl - 1:
                            # scale messages by sigma, scatter via one-hot rhs
                            xs = wpool.tile([P, CH * co_max], BF16, tag="xs")
                            nc.vector.tensor_tensor(
                                out=xs[:, 0 : hn * co].rearrange(
                                    "p (j d) -> p j d", d=co
                                ),
                                in0=g3[:, :, 0:co],
                                in1=ssig[:, 0:hn]
                                .rearrange("p (j o) -> p j o", o=1)
                                .to_broadcast([P, hn, co]),
                                op=ALU.mult,
                            )
                            for u in range(hn):
                                nc.tensor.matmul(
                                    out=o_ps,
                                    lhsT=xs[:, u * co : (u + 1) * co],
                                    rhs=oh3[:, u, :],
                                    start=first_mm,
                                    stop=last_ch and (u == hn - 1),
                                )
                                first_mm = False
                        else:
                            # wide co: cheaper to scale the one-hot by sigma
                            ohs = wpool.tile([P, CH * P], BF16, tag="ohs")
                            ohs3 = ohs[:, 0 : hn * P].rearrange(
                                "p (j s) -> p j s", s=P
                            )
                            nc.vector.tensor_tensor(
                                out=ohs3,
                                in0=oh3,
                                in1=ssig[:, 0:hn]
                                .rearrange("p (j o) -> p j o", o=1)
                                .to_broadcast([P, hn, P]),
                                op=ALU.mult,
                            )
                            for u in range(hn):
                                nc.tensor.matmul(
                                    out=o_ps,
                                    lhsT=ohs3[:, u, :],
                                    rhs=g3[:, u, 0:co],
                                    start=first_mm,
                                    stop=last_ch and (u == hn - 1),
                                )
                                first_mm = False
                    if l < nl - 1:
                        nc.scalar.activation(
                            out=xloc[:, t * P : (t + 1) * P], in_=o_ps, func=AF.Relu
                        )
                    else:
                        ostg = wpool.tile([P, co], F32, tag="ostg")
                        nc.scalar.activation(out=ostg[:], in_=o_ps, func=AF.Copy)
                        nc.sync.dma_start(out_d[t * P : (t + 1) * P, :], ostg[:])

    nc.compile()
    return nc


# ---------------------------------------------------------------- driver

_PROG_CACHE = {}


def _make_in_maps(inputs, cfg, g_pad, per_core, x1t, fw):
    nbc, nl = cfg.nbc, len(cfg.dims)
    n_loc = nbc * P
    cf_cols = 3 + P
    cstf = np.zeros((P, cf_cols), np.float32)
    for l in range(nl):
        cstf[:, l] = fw[l]["b2"]
    cstf[0, 3 : 3 + P] = 1.0
    cb_cols = P + sum(d[1] for d in cfg.dims)
    cstb = np.zeros((P, cb_cols), NPBF)
    cstb[:, 0:P] = np.arange(P, dtype=np.float32)[None, :].astype(NPBF)
    off = P
    for l in range(nl):
        cstb[:, off : off + cfg.dims[l][1]] = fw[l]["w2"][None, :].astype(NPBF)
        off += cfg.dims[l][1]

    in_maps = []
    for c in range(NCORES):
        srcw, dstw, dstc_a = per_core[c]
        in_maps.append(
            {
                "xt": np.ascontiguousarray(x1t[:, c * n_loc : (c + 1) * n_loc]),
                "cstf": cstf,
                "cstb": cstb,
                "srcw": srcw,
                "dstw": dstw,
                "dstc": dstc_a.astype(NPBF),
                **{f"w{l + 1}": fw[l]["wmat"] for l in range(nl)},
            }
        )
    return in_maps


def _run(inputs, cfg, trace=False):
    x = np.ascontiguousarray(np.asarray(inputs["x"], dtype=np.float32))
    ei = np.asarray(inputs["edge_index"]).astype(np.int64)
    src, dst = ei[0], ei[1]

    g_of, g_pad, per_core, x1t = _host_prep(x, src, dst, cfg)
    fw = _fuse_weights(inputs, cfg)

    key = (cfg.n_real, cfg.nbc, g_pad)
    if key not in _PROG_CACHE:
        _PROG_CACHE[key] = _build_program(cfg, g_pad)
    nc = _PROG_CACHE[key]

    in_maps = _make_in_maps(inputs, cfg, g_pad, per_core, x1t, fw)
    res = run_bass_kernel_spmd(nc, in_maps, core_ids=list(range(NCORES)), trace=trace)

    n_loc = cfg.nbc * P
    full = np.empty((cfg.ng, cfg.dims[-1][1]), np.float32)
    for c in range(NCORES):
        full[c * n_loc : (c + 1) * n_loc] = res.results[c]["out"]
    out = full[g_of]
    return out, res


def kernel(**inputs) -> np.ndarray:
    out, _ = _run(inputs, CFG, trace=False)
    return out


# revision 22
# speedup vs baseline: 2.4957x; 1.3598x over previous
"""Trainium2 Bass kernel for a 3-layer PointGNN-style edge-scored message-passing GNN.

Per layer (host-folded weights):
    x' = X@W + b ; B = X@(W Wj) + b Wj ; A = X@(W Wi) + (b Wi + bs1)
    h = relu(A[dst] + B[src]) ; s = sigmoid(h.w2 + b2)
    out[d] = sum_{e: dst=d} s_e * x'[src_e]       (+relu for layers 1,2)

Device (8-core SPMD, dst-partitioned):
  - nodes permuted into nb=256 bins of 128 slots, balanced by in-degree; each
    bin's (edges+self-loops) list padded to uniform g_pad (16 tiles),
    dst-bin sorted. ng=32768 so node ids fit int16 gather indices.
  - node phase (own 32 bins, fp32 matmuls for precision): one chain per bin
    emits [x' | B | A]; [x'|B] rows (bf16) staged and AllGathered into a full
    DRAM table [ng, 2co]; A rows kept in a core-local DRAM table (dst is
    always core-local, so A never rides the collective).
  - edge phase (own 32 bins, chunks of 8 128-edge tiles): two dma_gathers per
    chunk ([x'|B] by src from the shared table, A by local dst); batched bf16
    DVE ops (one-hot, add, relu*w2, segmented reduce, sigma-scale) compute
    edge scores; one bf16 matmul per 128-edge tile scatter-adds weighted
    messages into the dst bin's PSUM accumulator (sigma folded into the
    messages for co=64 layers, into the one-hot for the wide co=256 layer).
"""

import sys

if "/opt/trn_rl_repo" not in sys.path:
    sys.path.insert(0, "/opt/trn_rl_repo")

import numpy as np
import ml_dtypes

import concourse.bacc as bacc
import concourse.bass as bass  # noqa: F401
import concourse.mybir as mybir
import concourse.tile as tile
from concourse.bass_utils import run_bass_kernel_spmd

F32 = mybir.dt.float32
BF16 = mybir.dt.bfloat16
I16 = mybir.dt.int16
AF = mybir.ActivationFunctionType
ALU = mybir.AluOpType
NPBF = ml_dtypes.bfloat16

P = 128
NCORES = 8
CH = 8          # tiles (128 edges each) per gather chunk / compute group
SIM_MODE = False  # replace collectives with local copies (TimelineSim support)


def _a_w(co):
    """Local A-table row width (elements): A padded to a 256B-multiple stride."""
    w = co
    pad = (-w * 2) % 256  # bf16 bytes
    return w + pad // 2


class Cfg:
    def __init__(self, n_real, nbc, dims):
        self.n_real = n_real
        self.nbc = nbc
        self.nb = nbc * NCORES
        self.ng = self.nb * P
        self.dims = dims


CFG = Cfg(30000, 32, [(256, 64), (64, 64), (64, 256)])


# ---------------------------------------------------------------- host prep

def _balance_bins(weight, nb):
    """Assign nodes to nb bins of <=128 slots, balancing sum(weight)."""
    import heapq

    n = weight.shape[0]
    order = np.argsort(-weight, kind="stable")
    bin_of = np.empty(n, np.int32)
    slot_of = np.empty(n, np.int32)
    counts = np.zeros(nb, np.int32)
    heap = [(0, b) for b in range(nb)]
    heapq.heapify(heap)
    for i in order:
        spill = []
        while True:
            load, b = heapq.heappop(heap)
            if counts[b] < P:
                break
            spill.append((load, b))
        for s in spill:
            heapq.heappush(heap, s)
        bin_of[i] = b
        slot_of[i] = counts[b]
        counts[b] += 1
        heapq.heappush(heap, (load + int(weight[i]), b))
    return bin_of, slot_of


def _wrap16(flat_idx):
    n = flat_idx.shape[0]
    a = flat_idx.reshape(n // 16, 16).T.astype(np.int16)
    return np.tile(a, (8, 1))


def _host_prep(x, src, dst, cfg):
    n = cfg.n_real
    loops = np.arange(n, dtype=np.int64)
    src_all = np.concatenate([src, loops])
    dst_all = np.concatenate([dst, loops])

    indeg = np.bincount(dst_all, minlength=n).astype(np.int64)
    bin_of, slot_of = _balance_bins(indeg, cfg.nb)
    g_of = bin_of.astype(np.int64) * P + slot_of

    e_bin = bin_of[dst_all]
    order = np.argsort(e_bin, kind="stable")
    sb = e_bin[order]
    counts = np.bincount(e_bin, minlength=cfg.nb)
    g_pad = int(np.ceil(max(counts.max(), 1) / P) * P)
    starts = np.zeros(cfg.nb, np.int64)
    starts[1:] = np.cumsum(counts)[:-1]
    rank = np.arange(sb.shape[0]) - starts[sb]

    src_g = np.zeros((cfg.nb, g_pad), np.int64)             # pad edges -> row 0
    dst_g = np.zeros((cfg.nb, g_pad), np.int64)             # pad edges -> row 0
    dst_slot = np.full((cfg.nb, g_pad), 255.0, np.float32)  # pad -> no match
    src_g[sb, rank] = g_of[src_all[order]]
    dst_g[sb, rank] = g_of[dst_all[order]]
    dst_slot[sb, rank] = slot_of[dst_all[order]].astype(np.float32)

    nt_e = g_pad // P
    per_core = []
    for c in range(NCORES):
        bins = slice(c * cfg.nbc, (c + 1) * cfg.nbc)
        sg = src_g[bins]
        dg = dst_g[bins] - c * cfg.nbc * P
        dg[dst_slot[bins] == 255.0] = 0
        srcw = np.concatenate([_wrap16(sg[t]) for t in range(cfg.nbc)], axis=1)
        dstw = np.concatenate([_wrap16(dg[t]) for t in range(cfg.nbc)], axis=1)
        dc = dst_slot[bins]
        dstc = np.concatenate(
            [dc[t].reshape(nt_e, P).T for t in range(cfg.nbc)], axis=1
        ).astype(np.float32)
        per_core.append((srcw, dstw, dstc))

    c_in = cfg.dims[0][0]
    x1t = np.zeros((c_in, cfg.ng), np.float32)
    x1t[:, g_of] = x.T
    return g_of, g_pad, per_core, x1t


def _fuse_weights(ws, cfg):
    out = []
    for li, (ci, co) in enumerate(cfg.dims, start=1):
        wl = ws[f"w_lin{li}"].astype(np.float64)
        bl = ws[f"b_lin{li}"].astype(np.float64)
        ws1 = ws[f"w_s1_{li}"].astype(np.float64)
        bs1 = ws[f"b_s1_{li}"].astype(np.float64)
        ws2 = ws[f"w_s2_{li}"].astype(np.float64)
        bs2 = ws[f"b_s2_{li}"].astype(np.float64)
        wi, wj = ws1[:co], ws1[co:]
        wmat = np.zeros((ci + 1, 3 * co), np.float32)
        wmat[:ci, :co] = wl
        wmat[ci, :co] = bl
        wmat[:ci, co : 2 * co] = wl @ wj
        wmat[ci, co : 2 * co] = bl @ wj
        wmat[:ci, 2 * co :] = wl @ wi
        wmat[ci, 2 * co :] = bl @ wi + bs1
        out.append(dict(wmat=wmat, w2=ws2[:, 0].astype(np.float32), b2=np.float32(bs2[0])))
    return out


# ---------------------------------------------------------------- program

def _build_program(cfg, g_pad):
    nbc, ng = cfg.nbc, cfg.ng
    nt_e = g_pad // P
    dims = cfg.dims
    nl = len(dims)
    c_in1 = dims[0][0]
    co_last = dims[-1][1]
    n_loc = nbc * P
    k1 = c_in1 // P
    chunks = [(j0, min(CH, nt_e - j0)) for j0 in range(0, nt_e, CH)]
    co_max = max(d[1] for d in dims)
    aw_max = max(_a_w(d[1]) for d in dims)

    # f32 const blob columns: b2 per layer + ones_row (row 0)
    c_b2 = [0, 1, 2]
    c_ones = 3
    cf_cols = 3 + P
    # bf16 const blob columns: iota row + w2rep per layer
    c_iota = 0
    cb_off = []
    off = P
    for l in range(nl):
        cb_off.append(off)
        off += dims[l][1]
    cb_cols = off

    nc = bacc.Bacc(
        "TRN2",
        target_bir_lowering=False,
        debug=False,
        num_devices=NCORES,
        num_swdge_queues=2,
    )

    xt_d = nc.dram_tensor("xt", [c_in1, n_loc], F32, kind="ExternalInput")
    cstf_d = nc.dram_tensor("cstf", [P, cf_cols], F32, kind="ExternalInput")
    cstb_d = nc.dram_tensor("cstb", [P, cb_cols], BF16, kind="ExternalInput")
    srcw_d = nc.dram_tensor("srcw", [P, nbc * g_pad // 16], I16, kind="ExternalInput")
    dstw_d = nc.dram_tensor("dstw", [P, nbc * g_pad // 16], I16, kind="ExternalInput")
    dstc_d = nc.dram_tensor("dstc", [P, nbc * nt_e], BF16, kind="ExternalInput")
    w_d = [
        nc.dram_tensor(f"w{l + 1}", [dims[l][0] + 1, 3 * dims[l][1]], F32, kind="ExternalInput")
        for l in range(nl)
    ]
    out_d = nc.dram_tensor("out", [n_loc, co_last], F32, kind="ExternalOutput")

    with tile.TileContext(nc) as tc:
        with (
            tc.tile_pool(name="cst", bufs=1) as cpool,
            tc.tile_pool(name="persist", bufs=1) as ppool,
            tc.tile_pool(name="stage", bufs=1) as spool,
            tc.tile_pool(name="work", bufs=3) as wpool,
            tc.tile_pool(name="edge", bufs=3) as epool,
            tc.tile_pool(name="gath", bufs=4) as gpool,
            tc.tile_pool(name="nps", bufs=2, space="PSUM") as npspool,
            tc.tile_pool(name="acc", bufs=2, space="PSUM") as accpool,
            tc.tile_pool(name="dram", bufs=1, space="DRAM") as dpool,
        ):
            # ---------------- constants
            cstf = cpool.tile([P, cf_cols], F32)
            nc.sync.dma_start(cstf[:], cstf_d[:])
            cstb = cpool.tile([P, cb_cols], BF16)
            nc.sync.dma_start(cstb[:], cstb_d[:])
            srcw = cpool.tile([P, nbc * g_pad // 16], I16)
            nc.sync.dma_start(srcw[:], srcw_d[:])
            dstw = cpool.tile([P, nbc * g_pad // 16], I16)
            nc.sync.dma_start(dstw[:], dstw_d[:])
            dstc = cpool.tile([P, nbc * nt_e], BF16)
            nc.sync.dma_start(dstc[:], dstc_d[:])
            wfeat = []
            wbias = []
            for l in range(nl):
                ci_l, co_l = dims[l]
                nk = (ci_l + P - 1) // P
                chs = []
                for k in range(nk):
                    r0, r1 = k * P, min((k + 1) * P, ci_l)
                    w_t = cpool.tile([r1 - r0, 3 * co_l], F32, tag=f"w{l}_{k}")
                    nc.sync.dma_start(w_t[:], w_d[l][:][r0:r1, :])
                    chs.append(w_t)
                wb = cpool.tile([1, 3 * co_l], F32, tag=f"w{l}_b")
                nc.sync.dma_start(wb[:], w_d[l][:][ci_l : ci_l + 1, :])
                wfeat.append(chs)
                wbias.append(wb)

            iota_free = cstb[:, c_iota : c_iota + P]
            ones_row = cstf[0:1, c_ones : c_ones + P]

            # ---------------- persistent SBUF
            xloc = ppool.tile([64, n_loc], F32, tag="xloc")
            SG = 8  # bins per staging group

            # ---------------- DRAM internals
            ag_in = [
                dpool.tile(
                    [n_loc, 2 * dims[l][1]], BF16, tag=f"agin{l}", name=f"agin{l}"
                )
                for l in range(nl)
            ]
            tables = [
                dpool.tile(
                    [ng, 2 * dims[l][1]],
                    BF16,
                    tag=f"table{l}",
                    name=f"table{l}",
                    addr_space="Local" if SIM_MODE else "Shared",
                )
                for l in range(nl)
            ]
            atabs = [
                dpool.tile(
                    [n_loc, _a_w(dims[l][1])], BF16, tag=f"atab{l}", name=f"atab{l}"
                )
                for l in range(nl)
            ]

            for l in range(nl):
                ci, co = dims[l]
                aw = _a_w(co)

                # ======== node phase: own bins -> [x'|B|A] rows
                for t in range(nbc):
                    if t % SG == 0:
                        tstage = spool.tile([P, SG * 2 * co_max], BF16, tag="tstage")
                        astage = spool.tile([P, SG * aw_max], BF16, tag="astage")
                    tg = t % SG
                    cols = slice(t * P, (t + 1) * P)
                    if l == 0:
                        xa = wpool.tile([P, k1 * P], F32, tag="xa")
                        xa3 = xa[:].rearrange("p (c n) -> p c n", c=k1)
                        nc.sync.dma_start(
                            xa3,
                            xt_d[:, cols].rearrange("(c p) n -> p c n", p=P),
                        )
                        kch = [(xa3[:, k, :], wfeat[l][k]) for k in range(k1)]
                    else:
                        kch = [(xloc[:, cols], wfeat[l][0])]
                    if 3 * co <= 512:
                        ps = npspool.tile([P, 512], F32, space="PSUM", tag="nps_a")
                        parts = [(ps, 0, 3 * co)]
                    else:
                        ps1 = npspool.tile([P, 512], F32, space="PSUM", tag="nps_a")
                        ps2 = npspool.tile([P, 256], F32, space="PSUM", tag="nps_b")
                        parts = [(ps1, 0, 2 * co), (ps2, 2 * co, 3 * co)]
                    for pst, cc0, cc1 in parts:
                        for k, (kc, wt) in enumerate(kch):
                            nc.tensor.matmul(
                                out=pst[:, 0 : cc1 - cc0],
                                lhsT=kc,
                                rhs=wt[:, cc0:cc1],
                                start=(k == 0),
                                stop=False,
                            )
                        nc.tensor.matmul(
                            out=pst[:, 0 : cc1 - cc0],
                            lhsT=ones_row,
                            rhs=wbias[l][0:1, cc0:cc1],
                            start=False,
                            stop=True,
                        )
                    ps_a, a_lo = (
                        (parts[0][0], 2 * co) if len(parts) == 1 else (parts[1][0], 0)
                    )
                    nc.scalar.activation(
                        out=tstage[:, tg * 2 * co : (tg + 1) * 2 * co],
                        in_=parts[0][0][:, 0 : 2 * co],
                        func=AF.Copy,
                    )
                    nc.scalar.activation(
                        out=astage[:, tg * aw : tg * aw + co],
                        in_=ps_a[:, a_lo : a_lo + co],
                        func=AF.Copy,
                    )
                    if aw > co:
                        nc.scalar.activation(
                            out=astage[:, tg * aw + co : (tg + 1) * aw],
                            in_=ps_a[:, a_lo : a_lo + co],
                            func=AF.Copy,
                        )
                    if t % SG == SG - 1:
                        r0 = (t - SG + 1) * P
                        nc.sync.dma_start(
                            ag_in[l][:][r0 : r0 + SG * P, :].rearrange(
                                "(t p) c -> p t c", p=P
                            ),
                            tstage[:, 0 : SG * 2 * co].rearrange(
                                "p (t c) -> p t c", c=2 * co
                            ),
                        )
                        nc.sync.dma_start(
                            atabs[l][:][r0 : r0 + SG * P, :].rearrange(
                                "(t p) c -> p t c", p=P
                            ),
                            astage[:, 0 : SG * aw].rearrange("p (t c) -> p t c", c=aw),
                        )

                # AllGather the staged [x'|B] rows into the full table
                if SIM_MODE:
                    for r in range(NCORES):
                        nc.sync.dma_start(
                            tables[l][:][r * n_loc : (r + 1) * n_loc, :], ag_in[l][:]
                        )
                else:
                    nc.gpsimd.collective_compute(
                        "AllGather",
                        ALU.bypass,
                        replica_groups=[list(range(NCORES))],
                        ins=[ag_in[l].opt()],
                        outs=[tables[l].opt()],
                    )

                # ======== edge phase: own bins
                w2rep = cstb[:, cb_off[l] : cb_off[l] + co]
                b2col = cstf[:, c_b2[l] : c_b2[l] + 1]
                for t in range(nbc):
                    o_full = accpool.tile([P, 256], F32, space="PSUM", tag="o_ps")
                    o_ps = o_full[0:64, 0:P] if l < nl - 1 else o_full[:, 0:co]
                    first_mm = True
                    for ci_ch, (j0, hn) in enumerate(chunks):
                        gt0 = t * nt_e + j0
                        gbuf = gpool.tile([P, CH * 2 * co], BF16, tag="gbuf")
                        g3 = gbuf[:, 0 : hn * 2 * co].rearrange(
                            "p (j d) -> p j d", d=2 * co
                        )
                        nc.gpsimd.dma_gather(
                            out_ap=g3,
                            in_ap=tables[l][:],
                            idxs_ap=srcw[:, gt0 * 8 : (gt0 + hn) * 8],
                            num_idxs=hn * P,
                            num_idxs_reg=hn * P,
                            elem_size=2 * co,
                            queue_num=ci_ch % 2,
                        )
                        abuf = gpool.tile([P, CH * aw], BF16, tag="abuf")
                        ag3 = abuf[:, 0 : hn * aw].rearrange(
                            "p (j d) -> p j d", d=aw
                        )
                        nc.gpsimd.dma_gather(
                            out_ap=ag3,
                            in_ap=atabs[l][:],
                            idxs_ap=dstw[:, gt0 * 8 : (gt0 + hn) * 8],
                            num_idxs=hn * P,
                            num_idxs_reg=hn * P,
                            elem_size=aw,
                            queue_num=(ci_ch + 1) % 2,
                        )
                        a3 = ag3[:, :, 0:co]
                        # one-hot [e, slot] per tile for the scatter
                        oh = epool.tile([P, CH * P], BF16, tag="oh")
                        oh3 = oh[:, 0 : hn * P].rearrange("p (j s) -> p j s", s=P)
                        nc.vector.tensor_tensor(
                            out=oh3,
                            in0=dstc[:, t * nt_e + j0 : t * nt_e + j0 + hn]
                            .rearrange("p (j o) -> p j o", o=1)
                            .to_broadcast([P, hn, P]),
                            in1=iota_free.rearrange("p (o s) -> p o s", o=1)
                            .to_broadcast([P, hn, P]),
                            op=ALU.is_equal,
                        )
                        # pre-activation, relu * w2, segmented row-sum
                        tmp = epool.tile([P, CH * co_max], BF16, tag="tmp")
                        t3 = tmp[:, 0 : hn * co].rearrange("p (j d) -> p j d", d=co)
                        nc.vector.tensor_tensor(
                            out=t3,
                            in0=a3,
                            in1=g3[:, :, co : 2 * co],
                            op=ALU.add,
                        )
                        tmp2 = epool.tile([P, CH * co_max], BF16, tag="tmp2")
                        nc.vector.scalar_tensor_tensor(
                            out=tmp2[:, 0 : hn * co].rearrange("p (j d) -> p j d", d=co),
                            in0=t3,
                            scalar=0.0,
                            in1=w2rep.rearrange("p (o d) -> p o d", o=1)
                            .to_broadcast([P, hn, co]),
                            op0=ALU.max,
                            op1=ALU.mult,
                        )
                        spre = wpool.tile([P, CH], F32, tag="spre")
                        nc.vector.tensor_reduce(
                            out=spre[:, 0:hn],
                            in_=tmp2[:, 0 : hn * co].rearrange("p (j d) -> p j d", d=co),
                            axis=mybir.AxisListType.X,
                            op=ALU.add,
                        )
                        ssig = wpool.tile([P, CH], BF16, tag="ssig")
                        nc.scalar.activation(
                            out=ssig[:, 0:hn],
                            in_=spre[:, 0:hn],
                            func=AF.Sigmoid,
                            bias=b2col,
                        )
                        last_ch = ci_ch == len(chunks) - 1
                        if l < nl - 1:
                            # scale messages by sigma, scatter via one-hot rhs
                            xs = epool.tile([P, CH * co_max], BF16, tag="xs")
                            nc.vector.tensor_tensor(
                                out=xs[:, 0 : hn * co].rearrange(
                                    "p (j d) -> p j d", d=co
                                ),
                                in0=g3[:, :, 0:co],
                                in1=ssig[:, 0:hn]
                                .rearrange("p (j o) -> p j o", o=1)
                                .to_broadcast([P, hn, co]),
                                op=ALU.mult,
                            )
                            for u in range(hn):
                                nc.tensor.matmul(
                                    out=o_ps,
                                    lhsT=xs[:, u * co : (u + 1) * co],
                                    rhs=oh3[:, u, :],
                                    start=first_mm,
                                    stop=last_ch and (u == hn - 1),
                                )
                                first_mm = False
                        else:
                            # wide co: cheaper to scale the one-hot by sigma
                            ohs = epool.tile([P, CH * P], BF16, tag="ohs")
                            ohs3 = ohs[:, 0 : hn * P].rearrange(
                                "p (j s) -> p j s", s=P
                            )
                            nc.vector.tensor_tensor(
                                out=ohs3,
                                in0=oh3,
                                in1=ssig[:, 0:hn]
                                .rearrange("p (j o) -> p j o", o=1)
                                .to_broadcast([P, hn, P]),
                                op=ALU.mult,
                            )
                            for u in range(hn):
                                nc.tensor.matmul(
                                    out=o_ps,
                                    lhsT=ohs3[:, u, :],
                                    rhs=g3[:, u, 0:co],
                                    start=first_mm,
                                    stop=last_ch and (u == hn - 1),
                                )
                                first_mm = False
                    if l < nl - 1:
                        nc.scalar.activation(
                            out=xloc[:, t * P : (t + 1) * P], in_=o_ps, func=AF.Relu
                        )
                    else:
                        ostg = wpool.tile([P, co], F32, tag="ostg")
                        nc.scalar.activation(out=ostg[:], in_=o_ps, func=AF.Copy)
                        nc.sync.dma_start(out_d[t * P : (t + 1) * P, :], ostg[:])

    nc.compile()
    return nc


# ---------------------------------------------------------------- driver

_PROG_CACHE = {}


def _make_in_maps(inputs, cfg, g_pad, per_core, x1t, fw):
    nbc, nl = cfg.nbc, len(cfg.dims)
    n_loc = nbc * P
    cf_cols = 3 + P
    cstf = np.zeros((P, cf_cols), np.float32)
    for l in range(nl):
        cstf[:, l] = fw[l]["b2"]
    cstf[0, 3 : 3 + P] = 1.0
    cb_cols = P + sum(d[1] for d in cfg.dims)
    cstb = np.zeros((P, cb_cols), NPBF)
    cstb[:, 0:P] = np.arange(P, dtype=np.float32)[None, :].astype(NPBF)
    off = P
    for l in range(nl):
        cstb[:, off : off + cfg.dims[l][1]] = fw[l]["w2"][None, :].astype(NPBF)
        off += cfg.dims[l][1]

    in_maps = []
    for c in range(NCORES):
        srcw, dstw, dstc_a = per_core[c]
        in_maps.append(
            {
                "xt": np.ascontiguousarray(x1t[:, c * n_loc : (c + 1) * n_loc]),
                "cstf": cstf,
                "cstb": cstb,
                "srcw": srcw,
                "dstw": dstw,
                "dstc": dstc_a.astype(NPBF),
                **{f"w{l + 1}": fw[l]["wmat"] for l in range(nl)},
            }
        )
    return in_maps


def _run(inputs, cfg, trace=False):
    x = np.ascontiguousarray(np.asarray(inputs["x"], dtype=np.float32))
    ei = np.asarray(inputs["edge_index"]).astype(np.int64)
    src, dst = ei[0], ei[1]

    g_of, g_pad, per_core, x1t = _host_prep(x, src, dst, cfg)
    fw = _fuse_weights(inputs, cfg)

    key = (cfg.n_real, cfg.nbc, g_pad)
    if key not in _PROG_CACHE:
        _PROG_CACHE[key] = _build_program(cfg, g_pad)
    nc = _PROG_CACHE[key]

    in_maps = _make_in_maps(inputs, cfg, g_pad, per_core, x1t, fw)
    res = run_bass_kernel_spmd(nc, in_maps, core_ids=list(range(NCORES)), trace=trace)

    n_loc = cfg.nbc * P
    full = np.empty((cfg.ng, cfg.dims[-1][1]), np.float32)
    for c in range(NCORES):
        full[c * n_loc : (c + 1) * n_loc] = res.results[c]["out"]
    out = full[g_of]
    return out, res


def kernel(**inputs) -> np.ndarray:
    out, _ = _run(inputs, CFG, trace=False)
    return out
